# revision 6
# baseline (speedup 1.0000x reference)
"""Grouped MoE MLP (SwiGLU) for TRN2, expert-parallel across 8 NeuronCores.

Problem: T=8192 tokens pre-permuted into 8 contiguous expert segments of 1024,
H=1024, I=2816, per-expert weights gate/up [H,I], down [I,H].
    o1 = x @ gate; o2 = x @ up; h = silu(o1)*o2; out = h @ down

Sharding: expert-parallel — core e computes expert e's segment entirely
(zero collectives). Host slices inputs per expert and concatenates outputs.

Device kernel layout (per core), all matmuls in float32r (tf32-like, full
1-cycle/row rate at N=512, ~1e-4 rel err):
  - x fed host-transposed: xt [H, TE] so H (contraction) is on partitions.
  - stage 1: per I-slab of 128: o1T/o2T [128i, TE] = gate/up-slab.T @ xt,
    PSUM-accumulated over 8 H-chunks; SwiGLU fused: hT = silu(o1T)*o2T
    kept in SBUF ([I, TE], 22 slabs).
  - stage 2: out[TE, H] = hT.T @ down, PSUM-accumulated over 22 I-slabs,
    per (m-tile 128 tokens, h-chunk 512).

Weights are host-rearranged so every DMA is a fully-contiguous block.
"""

import os
import numpy as np
from contextlib import ExitStack

E, H, I, T = 8, 1024, 2816, 8192
TE = T // E  # tokens per expert = 1024
KC = H // 128  # 8 h-chunks
IS = I // 128  # 22 i-slabs
NCH = 512  # moving free dim per matmul (one PSUM bank of fp32)

_cache: dict = {}


def _build_nc(dt_tag: str):
    from concourse import bacc
    import concourse.tile as tile
    import concourse.mybir as mybir
    from concourse.bass import ts

    f32 = mybir.dt.float32
    dt = {"f32r": mybir.dt.float32r, "bf16": mybir.dt.bfloat16}[dt_tag]

    nc = bacc.Bacc("TRN2", target_bir_lowering=False, debug=False, num_devices=8)
    xt_d = nc.dram_tensor("xt", [KC, 128, TE], dt, kind="ExternalInput").ap()
    gate_d = nc.dram_tensor("gate", [IS, 128, KC, 128], dt, kind="ExternalInput").ap()
    up_d = nc.dram_tensor("up", [IS, 128, KC, 128], dt, kind="ExternalInput").ap()
    down_d = nc.dram_tensor("down", [IS, 128, H], dt, kind="ExternalInput").ap()
    out_d = nc.dram_tensor("out", [TE, H], f32, kind="ExternalOutput").ap()

    silu_fn = mybir.ActivationFunctionType.Silu

    with tile.TileContext(nc) as tc, ExitStack() as ctx:
        xt_pool = ctx.enter_context(tc.tile_pool(name="xt", bufs=2 * KC))
        g_pool = ctx.enter_context(tc.tile_pool(name="g", bufs=2))
        u_pool = ctx.enter_context(tc.tile_pool(name="u", bufs=2))
        h_pool = ctx.enter_context(tc.tile_pool(name="h", bufs=IS))
        d_pool = ctx.enter_context(tc.tile_pool(name="d", bufs=IS + 1))
        s_pool = ctx.enter_context(tc.tile_pool(name="s", bufs=2))
        o_pool = ctx.enter_context(tc.tile_pool(name="o", bufs=2))
        ps1 = ctx.enter_context(tc.tile_pool(name="ps1", bufs=2, space="PSUM"))
        ps3 = ctx.enter_context(tc.tile_pool(name="ps3", bufs=2, space="PSUM"))

        # resident xt half-tiles [128h, NCH], split by t-chunk so the first
        # matmuls' dependencies are small; DMA emission order puts slab-0
        # gate + the tc0 halves first to minimize PE startup latency.
        gs, us = {}, {}
        # slab-0 gate/up split into two half-DMAs so the very first matmuls
        # are gated on ~256KB, not 512KB
        # wave 1 (exactly 8 DMAs -> 8 queues): g0 first half, xt tc0 halves
        # k=0..5, g0 second half. Covers MMs 0-5 plus the k>=4 gate tiles.
        gs[0] = g_pool.tile([128, KC, 128], dt, tag="g", name="g0")
        nc.sync.dma_start(out=gs[0][:, 0 : KC // 2, :], in_=gate_d[0, :, 0 : KC // 2])
        xth = [[None] * KC for _ in range(TE // NCH)]
        for k in range(6):
            t = xt_pool.tile([128, NCH], dt, tag="xt", name=f"xt0_{k}")
            nc.sync.dma_start(out=t[:], in_=xt_d[k, :, ts(0, NCH)])
            xth[0][k] = t
        nc.sync.dma_start(
            out=gs[0][:, KC // 2 : KC, :], in_=gate_d[0, :, KC // 2 : KC]
        )
        # wave 2: last two xt tc0 halves, u0 halves, then xt tc1 halves
        for k in (6, 7):
            t = xt_pool.tile([128, NCH], dt, tag="xt", name=f"xt0_{k}")
            nc.sync.dma_start(out=t[:], in_=xt_d[k, :, ts(0, NCH)])
            xth[0][k] = t
        us[0] = u_pool.tile([128, KC, 128], dt, tag="u", name="u0")
        nc.sync.dma_start(out=us[0][:, 0 : KC // 2, :], in_=up_d[0, :, 0 : KC // 2])
        nc.sync.dma_start(out=us[0][:, KC // 2 : KC, :], in_=up_d[0, :, KC // 2 : KC])
        for k in range(KC):
            t = xt_pool.tile([128, NCH], dt, tag="xt", name=f"xt1_{k}")
            nc.sync.dma_start(out=t[:], in_=xt_d[k, :, ts(1, NCH)])
            xth[1][k] = t

        # stage 1+2: per i-slab, o1T/o2T then fused SwiGLU into resident hT
        hts = []
        for i in range(IS):
            if i not in gs:
                gs[i] = g_pool.tile([128, KC, 128], dt, tag="g", name=f"g{i}")
                nc.sync.dma_start(out=gs[i][:], in_=gate_d[i])
                us[i] = u_pool.tile([128, KC, 128], dt, tag="u", name=f"u{i}")
                nc.sync.dma_start(out=us[i][:], in_=up_d[i])
            g, u = gs[i], us[i]
            ht = h_pool.tile([128, TE], dt, tag="h")
            for tci in range(TE // NCH):
                p1 = ps1.tile([128, NCH], f32, tag="p1")
                p2 = ps1.tile([128, NCH], f32, tag="p2")
                for k in range(KC):
                    nc.tensor.matmul(
                        p1[:],
                        lhsT=g[:, k, :],
                        rhs=xth[tci][k][:],
                        start=(k == 0),
                        stop=(k == KC - 1),
                    )
                for k in range(KC):
                    nc.tensor.matmul(
                        p2[:],
                        lhsT=u[:, k, :],
                        rhs=xth[tci][k][:],
                        start=(k == 0),
                        stop=(k == KC - 1),
                    )
                sl = s_pool.tile([128, NCH], f32, tag="s")
                nc.scalar.activation(sl[:], p1[:], silu_fn)
                nc.vector.tensor_mul(ht[:, ts(tci, NCH)], sl[:], p2[:])
            hts.append(ht)

        # stage 3: out[m,hc] = sum_i hT_i[:, m].T @ down_i[:, hc]
        # hc=1's first 16 d-tiles go in the xt tag: xt tiles are dead after
        # stage 1, so those slots free mid-stage-3-hc0 and the hc=1 loads
        # prefetch instead of stalling on d-slot releases at the hc boundary.
        for hc in range(H // NCH):
            dts = []
            for i in range(IS):
                pool_tag = "xt" if (hc == 1 and i < 2 * KC) else "d"
                d = d_pool.tile(
                    [128, NCH], dt, tag=pool_tag, name=f"d{hc}_{i}"
                ) if pool_tag == "d" else xt_pool.tile(
                    [128, NCH], dt, tag="xt", name=f"d{hc}_{i}"
                )
                nc.sync.dma_start(out=d[:], in_=down_d[i, :, ts(hc, NCH)])
                dts.append(d)
            for m in range(TE // 128):
                po = ps3.tile([128, NCH], f32, tag="po")
                for i in range(IS):
                    nc.tensor.matmul(
                        po[:],
                        lhsT=hts[i][:, ts(m, 128)],
                        rhs=dts[i][:],
                        start=(i == 0),
                        stop=(i == IS - 1),
                    )
                ob = o_pool.tile([128, NCH], f32, tag="o")
                nc.vector.tensor_copy(ob[:], po[:])
                nc.sync.dma_start(out=out_d[ts(m, 128), ts(hc, NCH)], in_=ob[:])

    nc.compile()
    return nc


def _get_nc(dt_tag: str):
    if dt_tag not in _cache:
        _cache[dt_tag] = _build_nc(dt_tag)
    return _cache[dt_tag]


def _prep_in_maps(x, gate, up, down, dt_tag: str):
    """Slice per expert and rearrange for contiguous device DMAs."""
    cast = None
    if dt_tag == "bf16":
        import ml_dtypes

        cast = ml_dtypes.bfloat16
    in_maps = []
    for e in range(E):
        xe = np.ascontiguousarray(x[e * TE : (e + 1) * TE].T)  # [H, TE]
        # gate/up [H, I] -> [IS, 128p(h%128), KC(h//128), 128m(i%128)]
        ge = np.ascontiguousarray(
            gate[e].reshape(KC, 128, IS, 128).transpose(2, 1, 0, 3)
        )
        ue = np.ascontiguousarray(up[e].reshape(KC, 128, IS, 128).transpose(2, 1, 0, 3))
        de = down[e].reshape(IS, 128, H)
        if cast is not None:
            xe, ge, ue, de = (a.astype(cast) for a in (xe, ge, ue, de))
        in_maps.append(
            {
                "xt": xe.reshape(KC, 128, TE),
                "gate": ge,
                "up": ue,
                "down": np.ascontiguousarray(de),
            }
        )
    return in_maps


def run(inputs: dict, trace: bool = False, tmpdir=None, dt_tag=None):
    """Full-input entry. Returns (output [T,H] f32, BassKernelResults|None)."""
    x = np.asarray(inputs["permuted_local_hidden_states"], dtype=np.float32)
    gate = np.asarray(inputs["grouped_gate_proj"], dtype=np.float32)
    up = np.asarray(inputs["grouped_up_proj"], dtype=np.float32)
    down = np.asarray(inputs["grouped_down_proj"], dtype=np.float32)
    tpe = np.asarray(inputs["tokens_per_expert"]).astype(np.int64)

    if not (x.shape == (T, H) and tpe.shape == (E,) and np.all(tpe == TE)):
        # general ragged fallback (host): correctness-only path
        out = np.empty((x.shape[0], down.shape[2]), dtype=np.float32)
        off = 0
        for e in range(E):
            n = int(tpe[e])
            xe = x[off : off + n]
            o1 = xe @ gate[e]
            o2 = xe @ up[e]
            hgl = (o1 / (1.0 + np.exp(-o1))) * o2
            out[off : off + n] = hgl @ down[e]
            off += n
        return out, None

    dt_tag = dt_tag or os.environ.get("BASS_MOE_DT", "f32r")
    from concourse.bass_utils import run_bass_kernel_spmd

    nc = _get_nc(dt_tag)
    in_maps = _prep_in_maps(x, gate, up, down, dt_tag)
    res = run_bass_kernel_spmd(
        nc, in_maps, list(range(E)), trace=trace, tmpdir=tmpdir
    )
    out = np.concatenate([res.results[e]["out"] for e in range(E)], axis=0)
    return out, res


def kernel(**inputs) -> np.ndarray:
    out, _ = run(inputs, trace=False)
    return out


# revision 7
# speedup vs baseline: 1.0834x; 1.0834x over previous
"""Grouped MoE MLP (SwiGLU) for TRN2, expert-parallel across 8 NeuronCores.

Problem: T=8192 tokens pre-permuted into 8 contiguous expert segments of 1024,
H=1024, I=2816, per-expert weights gate/up [H,I], down [I,H].
    o1 = x @ gate; o2 = x @ up; h = silu(o1)*o2; out = h @ down

Sharding: expert-parallel — core e computes expert e's segment entirely
(zero collectives). Host slices inputs per expert and concatenates outputs.

Device kernel layout (per core), all matmuls in float32r (tf32-like, full
1-cycle/row rate at N=512, ~1e-4 rel err):
  - x fed host-transposed: xt [H, TE] so H (contraction) is on partitions.
  - stage 1: per I-slab of 128: o1T/o2T [128i, TE] = gate/up-slab.T @ xt,
    PSUM-accumulated over 8 H-chunks; SwiGLU fused: hT = silu(o1T)*o2T
    kept in SBUF ([I, TE], 22 slabs).
  - stage 2: out[TE, H] = hT.T @ down, PSUM-accumulated over 22 I-slabs,
    per (m-tile 128 tokens, h-chunk 512).

Weights are host-rearranged so every DMA is a fully-contiguous block.
"""

import os
import numpy as np
from contextlib import ExitStack

E, H, I, T = 8, 1024, 2816, 8192
TE = T // E  # tokens per expert = 1024
KC = H // 128  # 8 h-chunks
IS = I // 128  # 22 i-slabs
NCH = 512  # moving free dim per matmul (one PSUM bank of fp32)

_cache: dict = {}


def _build_nc(dt_tag: str):
    from concourse import bacc
    import concourse.tile as tile
    import concourse.mybir as mybir
    from concourse.bass import ts

    f32 = mybir.dt.float32
    dt = {"f32r": mybir.dt.float32r, "bf16": mybir.dt.bfloat16}[dt_tag]

    nc = bacc.Bacc("TRN2", target_bir_lowering=False, debug=False, num_devices=8)
    xt_d = nc.dram_tensor("xt", [KC, 128, TE], dt, kind="ExternalInput").ap()
    gate_d = nc.dram_tensor("gate", [IS, 128, KC, 128], dt, kind="ExternalInput").ap()
    up_d = nc.dram_tensor("up", [IS, 128, KC, 128], dt, kind="ExternalInput").ap()
    down_d = nc.dram_tensor("down", [IS, 128, H], dt, kind="ExternalInput").ap()
    out_d = nc.dram_tensor("out", [TE, H], f32, kind="ExternalOutput").ap()

    silu_fn = mybir.ActivationFunctionType.Silu

    with tile.TileContext(nc) as tc, ExitStack() as ctx:
        xt_pool = ctx.enter_context(tc.tile_pool(name="xt", bufs=2 * KC))
        g_pool = ctx.enter_context(tc.tile_pool(name="g", bufs=2))
        u_pool = ctx.enter_context(tc.tile_pool(name="u", bufs=2))
        h_pool = ctx.enter_context(tc.tile_pool(name="h", bufs=IS))
        d_pool = ctx.enter_context(tc.tile_pool(name="d", bufs=IS + 1))
        s_pool = ctx.enter_context(tc.tile_pool(name="s", bufs=2))
        o_pool = ctx.enter_context(tc.tile_pool(name="o", bufs=2))
        ps1 = ctx.enter_context(tc.tile_pool(name="ps1", bufs=2, space="PSUM"))
        ps3 = ctx.enter_context(tc.tile_pool(name="ps3", bufs=2, space="PSUM"))

        # resident xt half-tiles [128h, NCH], split by t-chunk so the first
        # matmuls' dependencies are small; DMA emission order puts slab-0
        # gate + the tc0 halves first to minimize PE startup latency.
        gs, us = {}, {}
        # slab-0 gate/up split into two half-DMAs so the very first matmuls
        # are gated on ~256KB, not 512KB
        # wave 1 (exactly 8 DMAs -> 8 queues): g0 first half, xt tc0 halves
        # k=0..5, g0 second half. Covers MMs 0-5 plus the k>=4 gate tiles.
        gs[0] = g_pool.tile([128, KC, 128], dt, tag="g", name="g0")
        nc.sync.dma_start(out=gs[0][:, 0 : KC // 2, :], in_=gate_d[0, :, 0 : KC // 2])
        xth = [[None] * KC for _ in range(TE // NCH)]
        for k in range(6):
            t = xt_pool.tile([128, NCH], dt, tag="xt", name=f"xt0_{k}")
            nc.sync.dma_start(out=t[:], in_=xt_d[k, :, ts(0, NCH)])
            xth[0][k] = t
        nc.sync.dma_start(
            out=gs[0][:, KC // 2 : KC, :], in_=gate_d[0, :, KC // 2 : KC]
        )
        # wave 2: last two xt tc0 halves, u0 halves, then xt tc1 halves
        for k in (6, 7):
            t = xt_pool.tile([128, NCH], dt, tag="xt", name=f"xt0_{k}")
            nc.sync.dma_start(out=t[:], in_=xt_d[k, :, ts(0, NCH)])
            xth[0][k] = t
        us[0] = u_pool.tile([128, KC, 128], dt, tag="u", name="u0")
        nc.sync.dma_start(out=us[0][:, 0 : KC // 2, :], in_=up_d[0, :, 0 : KC // 2])
        nc.sync.dma_start(out=us[0][:, KC // 2 : KC, :], in_=up_d[0, :, KC // 2 : KC])
        for k in range(KC):
            t = xt_pool.tile([128, NCH], dt, tag="xt", name=f"xt1_{k}")
            nc.sync.dma_start(out=t[:], in_=xt_d[k, :, ts(1, NCH)])
            xth[1][k] = t

        # stage 1+2: per i-slab, o1T/o2T then fused SwiGLU into resident hT
        hts = []
        for i in range(IS):
            if i not in gs:
                gs[i] = g_pool.tile([128, KC, 128], dt, tag="g", name=f"g{i}")
                nc.sync.dma_start(out=gs[i][:], in_=gate_d[i])
                us[i] = u_pool.tile([128, KC, 128], dt, tag="u", name=f"u{i}")
                nc.sync.dma_start(out=us[i][:], in_=up_d[i])
            g, u = gs[i], us[i]
            ht = h_pool.tile([128, TE], dt, tag="h")
            for tci in range(TE // NCH):
                p1 = ps1.tile([128, NCH], f32, tag="p1")
                p2 = ps1.tile([128, NCH], f32, tag="p2")
                for k in range(KC):
                    nc.tensor.matmul(
                        p1[:],
                        lhsT=g[:, k, :],
                        rhs=xth[tci][k][:],
                        start=(k == 0),
                        stop=(k == KC - 1),
                    )
                for k in range(KC):
                    nc.tensor.matmul(
                        p2[:],
                        lhsT=u[:, k, :],
                        rhs=xth[tci][k][:],
                        start=(k == 0),
                        stop=(k == KC - 1),
                    )
                sl = s_pool.tile([128, NCH], f32, tag="s")
                nc.scalar.activation(sl[:], p1[:], silu_fn)
                nc.vector.tensor_mul(ht[:, ts(tci, NCH)], sl[:], p2[:])
            hts.append(ht)

        # stage 3: out[m,hc] = sum_i hT_i[:, m].T @ down_i[:, hc]
        # hc=1's first 16 d-tiles go in the xt tag: xt tiles are dead after
        # stage 1, so those slots free mid-stage-3-hc0 and the hc=1 loads
        # prefetch instead of stalling on d-slot releases at the hc boundary.
        for hc in range(H // NCH):
            dts = []
            for i in range(IS):
                pool_tag = "xt" if (hc == 1 and i < 2 * KC) else "d"
                d = d_pool.tile(
                    [128, NCH], dt, tag=pool_tag, name=f"d{hc}_{i}"
                ) if pool_tag == "d" else xt_pool.tile(
                    [128, NCH], dt, tag="xt", name=f"d{hc}_{i}"
                )
                nc.sync.dma_start(out=d[:], in_=down_d[i, :, ts(hc, NCH)])
                dts.append(d)
            for m in range(TE // 128):
                po = ps3.tile([128, NCH], f32, tag="po")
                for i in range(IS):
                    nc.tensor.matmul(
                        po[:],
                        lhsT=hts[i][:, ts(m, 128)],
                        rhs=dts[i][:],
                        start=(i == 0),
                        stop=(i == IS - 1),
                    )
                ob = o_pool.tile([128, NCH], f32, tag="o")
                nc.vector.tensor_copy(ob[:], po[:])
                nc.scalar.dma_start(out=out_d[ts(m, 128), ts(hc, NCH)], in_=ob[:])

    nc.compile()
    return nc


def _get_nc(dt_tag: str):
    if dt_tag not in _cache:
        _cache[dt_tag] = _build_nc(dt_tag)
    return _cache[dt_tag]


def _prep_in_maps(x, gate, up, down, dt_tag: str):
    """Slice per expert and rearrange for contiguous device DMAs."""
    cast = None
    if dt_tag == "bf16":
        import ml_dtypes

        cast = ml_dtypes.bfloat16
    in_maps = []
    for e in range(E):
        xe = np.ascontiguousarray(x[e * TE : (e + 1) * TE].T)  # [H, TE]
        # gate/up [H, I] -> [IS, 128p(h%128), KC(h//128), 128m(i%128)]
        ge = np.ascontiguousarray(
            gate[e].reshape(KC, 128, IS, 128).transpose(2, 1, 0, 3)
        )
        ue = np.ascontiguousarray(up[e].reshape(KC, 128, IS, 128).transpose(2, 1, 0, 3))
        de = down[e].reshape(IS, 128, H)
        if cast is not None:
            xe, ge, ue, de = (a.astype(cast) for a in (xe, ge, ue, de))
        in_maps.append(
            {
                "xt": xe.reshape(KC, 128, TE),
                "gate": ge,
                "up": ue,
                "down": np.ascontiguousarray(de),
            }
        )
    return in_maps


def run(inputs: dict, trace: bool = False, tmpdir=None, dt_tag=None):
    """Full-input entry. Returns (output [T,H] f32, BassKernelResults|None)."""
    x = np.asarray(inputs["permuted_local_hidden_states"], dtype=np.float32)
    gate = np.asarray(inputs["grouped_gate_proj"], dtype=np.float32)
    up = np.asarray(inputs["grouped_up_proj"], dtype=np.float32)
    down = np.asarray(inputs["grouped_down_proj"], dtype=np.float32)
    tpe = np.asarray(inputs["tokens_per_expert"]).astype(np.int64)

    if not (x.shape == (T, H) and tpe.shape == (E,) and np.all(tpe == TE)):
        # general ragged fallback (host): correctness-only path
        out = np.empty((x.shape[0], down.shape[2]), dtype=np.float32)
        off = 0
        for e in range(E):
            n = int(tpe[e])
            xe = x[off : off + n]
            o1 = xe @ gate[e]
            o2 = xe @ up[e]
            hgl = (o1 / (1.0 + np.exp(-o1))) * o2
            out[off : off + n] = hgl @ down[e]
            off += n
        return out, None

    dt_tag = dt_tag or os.environ.get("BASS_MOE_DT", "f32r")
    from concourse.bass_utils import run_bass_kernel_spmd

    nc = _get_nc(dt_tag)
    in_maps = _prep_in_maps(x, gate, up, down, dt_tag)
    res = run_bass_kernel_spmd(
        nc, in_maps, list(range(E)), trace=trace, tmpdir=tmpdir
    )
    out = np.concatenate([res.results[e]["out"] for e in range(E)], axis=0)
    return out, res


def kernel(**inputs) -> np.ndarray:
    out, _ = run(inputs, trace=False)
    return out
